# revision 1
# baseline (speedup 1.0000x reference)
import jax
import jax.numpy as jnp
import numpy as np

# Problem constants (hardcoded; kernel.py must be self-contained)
B, DIM, E = 32768, 4096, 256
G, TOPK_GROUPS, TOPK = 8, 4, 8
ROUTE_SCALE = 2.5
N_CORES = 8

_EG = E // G  # experts per group


def _route_shard(x, W, b):
    # x: [B/N_CORES, DIM], W: [E, DIM], b: [E]
    Bs = x.shape[0]
    scores = jax.nn.sigmoid(jnp.einsum('bd,ed->be', x, W) + b)
    original_scores = scores

    s = scores.reshape(Bs, G, _EG)
    group_scores = jax.lax.top_k(s, 2)[0].sum(-1)
    grp_idx = jax.lax.top_k(group_scores, TOPK_GROUPS)[1]

    keep = jnp.zeros((Bs, G), dtype=bool).at[jnp.arange(Bs)[:, None], grp_idx].set(True)
    s = jnp.where(keep[:, :, None], s, -jnp.inf).reshape(Bs, E)

    indices = jax.lax.top_k(s, TOPK)[1]
    weights = jnp.take_along_axis(original_scores, indices, axis=1)
    weights = weights / weights.sum(-1, keepdims=True)
    weights = (weights * ROUTE_SCALE).astype(x.dtype)
    return weights, indices


_pmapped = jax.pmap(_route_shard, in_axes=(0, None, None))


def kernel(x, W, b):
    x = np.asarray(x, dtype=np.float32)
    W = np.asarray(W, dtype=np.float32)
    b = np.asarray(b, dtype=np.float32)

    n = min(N_CORES, jax.device_count())
    xs = x.reshape(n, B // n, DIM)
    w_out, i_out = _pmapped(xs, W, b)
    weights = np.asarray(w_out).reshape(B, TOPK).astype(np.float32)
    indices = np.asarray(i_out).reshape(B, TOPK).astype(np.int32)
    return weights, indices
